# revision 1
# baseline (speedup 1.0000x reference)
"""Trainium2 Bass kernel for the differentiable LogicLayer forward pass.

Math (per output neuron j with a = x[:, idx_a[j]], b = x[:, idx_b[j]]):
    w      = softmax(weights[j])          # [14]
    coeffs = w @ OP_COEFFS                # [4] -> c0, ca, cb, cab
    out[:, j] = c0 + ca*a + cb*b + cab*a*b

Sharding: data-parallel over batch across 8 NeuronCores (1024 rows each);
weights / indices replicated.  Per core the kernel works feature-major:
partition p of an SBUF tile holds output neuron j = t*128 + p, the free dim
holds the 1024-sample batch shard.  The gathers x[:, idx] become row
gathers of the transposed shard xT[4096, 1024], done with the SWDGE
dma_gather (16 SDMA engines, 4 KiB/descriptor).  The softmax+collapse of
the tiny weights runs on-device (ACT exp + DVE reductions).  Outputs are
written transposed ([16384, 1024] per core) and untransposed on the host.
"""

import sys

import numpy as np

try:  # the axon sitecustomize usually provides concourse already
    import concourse  # noqa: F401
except ImportError:  # pragma: no cover
    sys.path.insert(0, "/opt/trn_rl_repo")

import concourse.bacc as bacc
import concourse.mybir as mybir
import concourse.tile as tile
from concourse.bass_utils import run_bass_kernel_spmd
from concourse.library_config import mlp as mlp_library

F32 = mybir.dt.float32
I16 = mybir.dt.int16

NCORES = 8
BATCH, IN_DIM, OUT_DIM, NOPS = 8192, 4096, 16384, 14
B = BATCH // NCORES            # 1024 batch rows per core
NJC = 512                      # output neurons per gather chunk
NCH = OUT_DIM // NJC           # 32 chunks
SL = NJC // 128                # 4 partition-slices per chunk
NT = OUT_DIM // 128            # 128 coefficient columns

_OP_COEFFS = np.array([
    [0,  0,  0,  1],
    [0,  1,  0, -1],
    [0,  1,  0,  0],
    [0,  0,  1, -1],
    [0,  0,  1,  0],
    [0,  1,  1, -2],
    [0,  1,  1, -1],
    [1, -1, -1,  1],
    [1, -1, -1,  2],
    [1,  0, -1,  0],
    [1,  0, -1,  1],
    [1, -1,  0,  0],
    [1, -1,  0,  1],
    [1,  0,  0, -1],
], dtype=np.float32)


def build_program():
    """Build + compile the per-core Bass program (identical on all cores)."""
    nc = bacc.Bacc("TRN2", target_bir_lowering=False, debug=False,
                   num_devices=NCORES)

    xt = nc.dram_tensor("xt", [IN_DIM, B], F32, kind="ExternalInput")
    wre = nc.dram_tensor("wre", [128, NT, NOPS], F32, kind="ExternalInput")
    opc = nc.dram_tensor("opc", [128, 4, NT, NOPS], F32, kind="ExternalInput")
    idxa = nc.dram_tensor("idxa", [128, OUT_DIM // 16], I16, kind="ExternalInput")
    idxb = nc.dram_tensor("idxb", [128, OUT_DIM // 16], I16, kind="ExternalInput")
    out = nc.dram_tensor("out", [OUT_DIM, B], F32, kind="ExternalOutput")

    # out rows j = ci*NJC + s*128 + p  ->  [ci, p, s, b] view for stores
    out_r = out.ap().rearrange("(c s p) b -> c p s b", s=SL, p=128)

    mult = mybir.AluOpType.mult
    add = mybir.AluOpType.add
    ident = mybir.ActivationFunctionType.Identity
    expf = mybir.ActivationFunctionType.Exp

    with tile.TileContext(nc) as tc:
        nc.gpsimd.load_library(mlp_library)
        with (
            tc.tile_pool(name="const", bufs=1) as cpool,
            tc.tile_pool(name="coef", bufs=1) as kpool,
        ):
            ia_sb = cpool.tile([128, OUT_DIM // 16], I16)
            nc.sync.dma_start(ia_sb[:], idxa.ap())
            ib_sb = cpool.tile([128, OUT_DIM // 16], I16)
            nc.sync.dma_start(ib_sb[:], idxb.ap())

            # ---- coefficients: softmax over the 14 ops, collapsed to 4 ----
            with tc.tile_pool(name="init", bufs=1) as ipool:
                w_sb = ipool.tile([128, NT, NOPS], F32)
                nc.sync.dma_start(w_sb[:], wre.ap())
                opc_sb = ipool.tile([128, 4, NT, NOPS], F32)
                nc.sync.dma_start(opc_sb[:], opc.ap())

                e_sb = ipool.tile([128, NT, NOPS], F32)
                nc.scalar.activation(e_sb[:], w_sb[:], expf)
                ssum = ipool.tile([128, NT], F32)
                nc.vector.tensor_reduce(ssum[:], e_sb[:],
                                        mybir.AxisListType.X, add)
                rsum = ipool.tile([128, NT], F32)
                nc.vector.reciprocal(rsum[:], ssum[:])

                # coef[m]: [128, NT] with element (p, t) = coeff_m[t*128+p]
                coef = []
                for m in range(4):
                    tmp = ipool.tile([128, NT, NOPS], F32, tag="ctmp")
                    nc.vector.tensor_tensor(tmp[:], e_sb[:], opc_sb[:, m],
                                            op=mult)
                    cm = kpool.tile([128, NT], F32, tag=f"coef{m}")
                    nc.vector.tensor_reduce(cm[:], tmp[:],
                                            mybir.AxisListType.X, add)
                    nc.vector.tensor_tensor(cm[:], cm[:], rsum[:], op=mult)
                    coef.append(cm)
                c0, ca, cb, cab = coef

            # ---- main loop: gather a/b rows, combine, store ----
            with (
                tc.tile_pool(name="ga", bufs=3) as apool,
                tc.tile_pool(name="gb", bufs=3) as bpool,
                tc.tile_pool(name="go", bufs=2) as opool,
                tc.tile_pool(name="uv", bufs=4) as uvpool,
            ):
                w16 = NJC // 16  # idx columns per chunk
                for ci in range(NCH):
                    at = apool.tile([128, SL, B], F32)
                    bt = bpool.tile([128, SL, B], F32)
                    nc.gpsimd.dma_gather(
                        at[:], xt.ap(), ia_sb[:, ci * w16:(ci + 1) * w16],
                        NJC, NJC, B)
                    nc.gpsimd.dma_gather(
                        bt[:], xt.ap(), ib_sb[:, ci * w16:(ci + 1) * w16],
                        NJC, NJC, B)
                    ot = opool.tile([128, SL, B], F32)
                    for s in range(SL):
                        t = ci * SL + s
                        u = uvpool.tile([128, B], F32, tag="u")
                        v = uvpool.tile([128, B], F32, tag="v")
                        # u = cab*a + cb ; v = ca*a + c0  (per-partition s/b)
                        nc.scalar.activation(u[:], at[:, s], ident,
                                             bias=cb[:, t:t + 1],
                                             scale=cab[:, t:t + 1])
                        nc.scalar.activation(v[:], at[:, s], ident,
                                             bias=c0[:, t:t + 1],
                                             scale=ca[:, t:t + 1])
                        # out_s = u*b + v  (DVE)
                        nc.vector.tensor_tensor(u[:], u[:], bt[:, s], op=mult)
                        nc.vector.tensor_tensor(ot[:, s], u[:], v[:], op=add)
                    nc.sync.dma_start(out_r[ci], ot[:])

    nc.compile()
    return nc


_PROGRAM = None


def _get_program():
    global _PROGRAM
    if _PROGRAM is None:
        _PROGRAM = build_program()
    return _PROGRAM


def _wrap_idx(idx):
    """[OUT_DIM] int -> SWDGE-wrapped int16 [128, OUT_DIM//16].

    Per NJC-chunk c, columns [c*NJC//16:(c+1)*NJC//16] hold that chunk's
    indices with index i at (partition i%16, column i//16), replicated
    across the 8 groups of 16 partitions (one per Q7 core).
    """
    i16 = idx.astype(np.int16).reshape(NCH, NJC // 16, 16)
    w = i16.transpose(2, 0, 1).reshape(16, NCH * (NJC // 16))
    return np.ascontiguousarray(np.tile(w, (8, 1)))


def prepare_in_maps(x, weights, idx_a, idx_b):
    x = np.asarray(x, dtype=np.float32)
    weights = np.asarray(weights, dtype=np.float32)
    idx_a = np.asarray(idx_a)
    idx_b = np.asarray(idx_b)

    wre = np.ascontiguousarray(
        weights.reshape(NT, 128, NOPS).transpose(1, 0, 2))
    opc = np.ascontiguousarray(
        np.broadcast_to(_OP_COEFFS.T[None, :, None, :],
                        (128, 4, NT, NOPS))).astype(np.float32)
    ia = _wrap_idx(idx_a)
    ib = _wrap_idx(idx_b)

    in_maps = []
    for c in range(NCORES):
        xt = np.ascontiguousarray(x[c * B:(c + 1) * B].T)
        in_maps.append({"xt": xt, "wre": wre, "opc": opc,
                        "idxa": ia, "idxb": ib})
    return in_maps


def assemble_output(results):
    out = np.empty((BATCH, OUT_DIM), dtype=np.float32)
    for c in range(NCORES):
        out[c * B:(c + 1) * B] = results[c]["out"].T
    return out


def kernel(x, weights, idx_a, idx_b):
    nc = _get_program()
    in_maps = prepare_in_maps(x, weights, idx_a, idx_b)
    res = run_bass_kernel_spmd(nc, in_maps, list(range(NCORES)))
    return assemble_output(res.results)



# revision 2
# speedup vs baseline: 1.4904x; 1.4904x over previous
"""Trainium2 Bass kernel for the differentiable LogicLayer forward pass.

Math (per output neuron j with a = x[:, idx_a[j]], b = x[:, idx_b[j]]):
    w      = softmax(weights[j])          # [14]
    coeffs = w @ OP_COEFFS                # [4] -> c0, ca, cb, cab
    out[:, j] = c0 + ca*a + cb*b + cab*a*b

Sharding: data-parallel over batch across 8 NeuronCores (1024 rows each);
weights / indices replicated.  Per core the kernel works feature-major:
partition p of an SBUF tile holds output neuron j = t*128 + p, the free dim
holds the 1024-sample batch shard.  The gathers x[:, idx] become row
gathers of the transposed shard xT[4096, 1024], done with the SWDGE
dma_gather (16 SDMA engines).

x, the gathered a/b tiles, and the output are all fp16: x is uniform[0,1)
(exactly representable to 5e-4) and the 2e-2 rel-err budget dwarfs fp16
rounding, while DMA bytes halve (gathers 128->64 MiB, store 64->32 MiB per
core) and the DVE gets its 2x/4x 16-bit modes.  Per 128-neuron slice the
combine is 1 ACT op (v = ca*a + c0, scale/bias trick) + 3 DVE ops
(u = cab*a + cb via fused tensor_scalar, then u*b, then +v).  The
softmax+collapse of the tiny weights runs fp32 on-device.  Outputs are
written transposed ([16384, 1024] fp16 per core) and untransposed /
upcast on the host.
"""

import sys

import numpy as np

try:  # the axon sitecustomize usually provides concourse already
    import concourse  # noqa: F401
except ImportError:  # pragma: no cover
    sys.path.insert(0, "/opt/trn_rl_repo")

import concourse.bacc as bacc
import concourse.mybir as mybir
import concourse.tile as tile
from concourse.bass_utils import run_bass_kernel_spmd
from concourse.library_config import mlp as mlp_library

F32 = mybir.dt.float32
F16 = mybir.dt.float16
I16 = mybir.dt.int16

NCORES = 8
BATCH, IN_DIM, OUT_DIM, NOPS = 8192, 4096, 16384, 14
B = BATCH // NCORES            # 1024 batch rows per core
NJC = 1024                     # output neurons per gather chunk
NCH = OUT_DIM // NJC           # 16 chunks
SL = NJC // 128                # 8 partition-slices per chunk
NT = OUT_DIM // 128            # 128 coefficient columns

_OP_COEFFS = np.array([
    [0,  0,  0,  1],
    [0,  1,  0, -1],
    [0,  1,  0,  0],
    [0,  0,  1, -1],
    [0,  0,  1,  0],
    [0,  1,  1, -2],
    [0,  1,  1, -1],
    [1, -1, -1,  1],
    [1, -1, -1,  2],
    [1,  0, -1,  0],
    [1,  0, -1,  1],
    [1, -1,  0,  0],
    [1, -1,  0,  1],
    [1,  0,  0, -1],
], dtype=np.float32)


def build_program():
    """Build + compile the per-core Bass program (identical on all cores)."""
    nc = bacc.Bacc("TRN2", target_bir_lowering=False, debug=False,
                   num_devices=NCORES)

    xt = nc.dram_tensor("xt", [IN_DIM, B], F16, kind="ExternalInput")
    wre = nc.dram_tensor("wre", [128, NT, NOPS], F32, kind="ExternalInput")
    opc = nc.dram_tensor("opc", [128, 4, NT, NOPS], F32, kind="ExternalInput")
    idxa = nc.dram_tensor("idxa", [128, OUT_DIM // 16], I16, kind="ExternalInput")
    idxb = nc.dram_tensor("idxb", [128, OUT_DIM // 16], I16, kind="ExternalInput")
    out = nc.dram_tensor("out", [OUT_DIM, B], F16, kind="ExternalOutput")

    # out rows j = ci*NJC + s*128 + p  ->  [ci, p, s, b] view for stores
    out_r = out.ap().rearrange("(c s p) b -> c p s b", s=SL, p=128)

    mult = mybir.AluOpType.mult
    add = mybir.AluOpType.add
    ident = mybir.ActivationFunctionType.Identity
    expf = mybir.ActivationFunctionType.Exp

    with tile.TileContext(nc) as tc:
        nc.gpsimd.load_library(mlp_library)
        with (
            tc.tile_pool(name="const", bufs=1) as cpool,
            tc.tile_pool(name="coef", bufs=1) as kpool,
        ):
            ia_sb = cpool.tile([128, OUT_DIM // 16], I16)
            nc.sync.dma_start(ia_sb[:], idxa.ap())
            ib_sb = cpool.tile([128, OUT_DIM // 16], I16)
            nc.sync.dma_start(ib_sb[:], idxb.ap())

            # ---- coefficients: softmax over the 14 ops, collapsed to 4 ----
            with tc.tile_pool(name="init", bufs=1) as ipool:
                w_sb = ipool.tile([128, NT, NOPS], F32)
                nc.sync.dma_start(w_sb[:], wre.ap())
                opc_sb = ipool.tile([128, 4, NT, NOPS], F32)
                nc.sync.dma_start(opc_sb[:], opc.ap())

                e_sb = ipool.tile([128, NT, NOPS], F32)
                nc.scalar.activation(e_sb[:], w_sb[:], expf)
                ssum = ipool.tile([128, NT], F32)
                nc.vector.tensor_reduce(ssum[:], e_sb[:],
                                        mybir.AxisListType.X, add)
                rsum = ipool.tile([128, NT], F32)
                nc.vector.reciprocal(rsum[:], ssum[:])

                # coef[m]: [128, NT] with element (p, t) = coeff_m[t*128+p]
                coef = []
                for m in range(4):
                    tmp = ipool.tile([128, NT, NOPS], F32, tag="ctmp")
                    nc.vector.tensor_tensor(tmp[:], e_sb[:], opc_sb[:, m],
                                            op=mult)
                    cm = kpool.tile([128, NT], F32, tag=f"coef{m}")
                    nc.vector.tensor_reduce(cm[:], tmp[:],
                                            mybir.AxisListType.X, add)
                    nc.vector.tensor_tensor(cm[:], cm[:], rsum[:], op=mult)
                    coef.append(cm)
                c0, ca, cb, cab = coef

            # ---- main loop: gather a/b rows, combine, store ----
            with (
                tc.tile_pool(name="ga", bufs=3) as apool,
                tc.tile_pool(name="gb", bufs=3) as bpool,
                tc.tile_pool(name="go", bufs=2) as opool,
                tc.tile_pool(name="uv", bufs=4) as uvpool,
            ):
                w16 = NJC // 16  # idx columns per chunk
                for ci in range(NCH):
                    at = apool.tile([128, SL, B], F16)
                    bt = bpool.tile([128, SL, B], F16)
                    nc.gpsimd.dma_gather(
                        at[:], xt.ap(), ia_sb[:, ci * w16:(ci + 1) * w16],
                        NJC, NJC, B)
                    nc.gpsimd.dma_gather(
                        bt[:], xt.ap(), ib_sb[:, ci * w16:(ci + 1) * w16],
                        NJC, NJC, B)
                    ot = opool.tile([128, SL, B], F16)
                    for s in range(SL):
                        t = ci * SL + s
                        u = uvpool.tile([128, B], F16, tag="u")
                        v = uvpool.tile([128, B], F16, tag="v")
                        # v = ca*a + c0  (ACT scale/bias)
                        nc.scalar.activation(v[:], at[:, s], ident,
                                             bias=c0[:, t:t + 1],
                                             scale=ca[:, t:t + 1])
                        # u = cab*a + cb (DVE fused tensor_scalar, 4x fp16)
                        nc.vector.tensor_scalar(u[:], at[:, s],
                                                cab[:, t:t + 1],
                                                cb[:, t:t + 1],
                                                mult, add)
                        # out_s = u*b + v  (DVE 2x fp16)
                        nc.vector.tensor_tensor(u[:], u[:], bt[:, s], op=mult)
                        nc.vector.tensor_tensor(ot[:, s], u[:], v[:], op=add)
                    nc.sync.dma_start(out_r[ci], ot[:])

    nc.compile()
    return nc


_PROGRAM = None


def _get_program():
    global _PROGRAM
    if _PROGRAM is None:
        _PROGRAM = build_program()
    return _PROGRAM


def _wrap_idx(idx):
    """[OUT_DIM] int -> SWDGE-wrapped int16 [128, OUT_DIM//16].

    Per NJC-chunk c, columns [c*NJC//16:(c+1)*NJC//16] hold that chunk's
    indices with index i at (partition i%16, column i//16), replicated
    across the 8 groups of 16 partitions (one per Q7 core).
    """
    i16 = idx.astype(np.int16).reshape(NCH, NJC // 16, 16)
    w = i16.transpose(2, 0, 1).reshape(16, NCH * (NJC // 16))
    return np.ascontiguousarray(np.tile(w, (8, 1)))


def prepare_in_maps(x, weights, idx_a, idx_b):
    x = np.asarray(x, dtype=np.float32)
    weights = np.asarray(weights, dtype=np.float32)
    idx_a = np.asarray(idx_a)
    idx_b = np.asarray(idx_b)

    wre = np.ascontiguousarray(
        weights.reshape(NT, 128, NOPS).transpose(1, 0, 2))
    opc = np.ascontiguousarray(
        np.broadcast_to(_OP_COEFFS.T[None, :, None, :],
                        (128, 4, NT, NOPS))).astype(np.float32)
    ia = _wrap_idx(idx_a)
    ib = _wrap_idx(idx_b)

    in_maps = []
    for c in range(NCORES):
        xt = np.ascontiguousarray(x[c * B:(c + 1) * B].T).astype(np.float16)
        in_maps.append({"xt": xt, "wre": wre, "opc": opc,
                        "idxa": ia, "idxb": ib})
    return in_maps


def assemble_output(results):
    out = np.empty((BATCH, OUT_DIM), dtype=np.float32)
    for c in range(NCORES):
        out[c * B:(c + 1) * B] = results[c]["out"].T.astype(np.float32)
    return out


def kernel(x, weights, idx_a, idx_b):
    nc = _get_program()
    in_maps = prepare_in_maps(x, weights, idx_a, idx_b)
    res = run_bass_kernel_spmd(nc, in_maps, list(range(NCORES)))
    return assemble_output(res.results)


# revision 7
# speedup vs baseline: 1.7987x; 1.2068x over previous
"""Trainium2 Bass kernel for the differentiable LogicLayer forward pass.

Math (per output neuron j with a = x[:, idx_a[j]], b = x[:, idx_b[j]]):
    w      = softmax(weights[j])          # [14]
    coeffs = w @ OP_COEFFS                # [4] -> c0, ca, cb, cab
    out[:, j] = c0 + ca*a + cb*b + cab*a*b

Sharding: data-parallel over batch across 8 NeuronCores (1024 rows each);
weights / indices replicated.  Per core the kernel works feature-major:
partition p of an SBUF tile holds output neuron j = ci*1024 + s*128 + p,
the free dim holds the 1024-sample batch shard.  The gathers x[:, idx]
become row gathers of the transposed shard xT[4096, 1024] (fp16, 2 KiB
rows) with the SWDGE dma_gather (1024 idxs per call — the HW cap).

Everything is fp16: x is uniform[0,1) (exact to 5e-4) and the 2e-2
rel-err budget dwarfs fp16 rounding, while DMA bytes halve and the DVE
gets its 16-bit modes.  The softmax coefficient collapse runs on the
host (16384x14, trivial).  Per 128-neuron slice the combine is
    u = cab*a + cb  (ACT scale/bias)
    w = u*b         (DVE tensor_tensor, 2x fp16)
    o = (ca*a + c0) + w   — even slices: ACT v then DVE tt add;
                            odd slices: one fused DVE affine_then_add
(the even/odd split balances ACT vs DVE engine time).
The store uses a tile-contiguous DRAM scratch layout [ci, p, s, b]
(16 KiB DMA lines); the host unscrambles and upcasts.
"""

import sys

import numpy as np

try:  # the axon sitecustomize usually provides concourse already
    import concourse  # noqa: F401
except ImportError:  # pragma: no cover
    sys.path.insert(0, "/opt/trn_rl_repo")

import concourse.bacc as bacc
import concourse.mybir as mybir
import concourse.tile as tile
from concourse.bass_utils import run_bass_kernel_spmd
from concourse.library_config import mlp as mlp_library

F32 = mybir.dt.float32
F16 = mybir.dt.float16
I16 = mybir.dt.int16

NCORES = 8
BATCH, IN_DIM, OUT_DIM, NOPS = 8192, 4096, 16384, 14
B = BATCH // NCORES            # 1024 batch rows per core
NJC = 1024                     # output neurons per chunk (SWDGE 1024-idx cap)
NCH = OUT_DIM // NJC           # 16 chunks
SL = NJC // 128                # 8 partition-slices per chunk
NT = OUT_DIM // 128            # 128 coefficient columns

_OP_COEFFS = np.array([
    [0,  0,  0,  1],
    [0,  1,  0, -1],
    [0,  1,  0,  0],
    [0,  0,  1, -1],
    [0,  0,  1,  0],
    [0,  1,  1, -2],
    [0,  1,  1, -1],
    [1, -1, -1,  1],
    [1, -1, -1,  2],
    [1,  0, -1,  0],
    [1,  0, -1,  1],
    [1, -1,  0,  0],
    [1, -1,  0,  1],
    [1,  0,  0, -1],
], dtype=np.float32)


def build_program():
    """Build + compile the per-core Bass program (identical on all cores)."""
    nc = bacc.Bacc("TRN2", target_bir_lowering=False, debug=False,
                   num_devices=NCORES)

    xt = nc.dram_tensor("xt", [IN_DIM, B], F16, kind="ExternalInput")
    # coefficients for ACT scale/bias + DVE tensor_scalar;
    # element (p, m, t) = coeff_m[t*128 + p]
    cf32 = nc.dram_tensor("cf32", [128, 4, NT], F32, kind="ExternalInput")
    # SWDGE 16-partition-wrapped gather indices per NJC-chunk
    idxa = nc.dram_tensor("idxa", [128, OUT_DIM // 16], I16,
                          kind="ExternalInput")
    idxb = nc.dram_tensor("idxb", [128, OUT_DIM // 16], I16,
                          kind="ExternalInput")
    # tile-contiguous scratch layout: [ci, p, s, b]; host unscrambles.
    out = nc.dram_tensor("out", [NCH, 128, SL, B], F16, kind="ExternalOutput")
    out_r = out.ap()

    mult = mybir.AluOpType.mult
    add = mybir.AluOpType.add
    ident = mybir.ActivationFunctionType.Identity

    with tile.TileContext(nc) as tc:
        nc.gpsimd.load_library(mlp_library)
        with (
            tc.tile_pool(name="const", bufs=1) as cpool,
        ):
            ia_sb = cpool.tile([128, OUT_DIM // 16], I16)
            nc.sync.dma_start(ia_sb[:], idxa.ap())
            ib_sb = cpool.tile([128, OUT_DIM // 16], I16)
            nc.sync.dma_start(ib_sb[:], idxb.ap())
            c32 = cpool.tile([128, 4, NT], F32)
            nc.sync.dma_start(c32[:], cf32.ap())
            c0_32, ca_32 = c32[:, 0], c32[:, 1]
            cb_32, cab_32 = c32[:, 2], c32[:, 3]

            # ---- main loop: gather a+b rows, combine, store ----
            with (
                tc.tile_pool(name="ga", bufs=3) as apool,
                tc.tile_pool(name="gb", bufs=3) as bpool,
                tc.tile_pool(name="go", bufs=2) as opool,
                tc.tile_pool(name="uv", bufs=4) as uvpool,
            ):
                w16 = NJC // 16  # idx columns per chunk
                for ci in range(NCH):
                    at = apool.tile([128, SL, B], F16)
                    bt = bpool.tile([128, SL, B], F16)
                    nc.gpsimd.dma_gather(
                        at[:], xt.ap(), ia_sb[:, ci * w16:(ci + 1) * w16],
                        NJC, NJC, B)
                    nc.gpsimd.dma_gather(
                        bt[:], xt.ap(), ib_sb[:, ci * w16:(ci + 1) * w16],
                        NJC, NJC, B)
                    ot = opool.tile([128, SL, B], F16)
                    for s in range(SL):
                        t = ci * SL + s
                        a_s = at[:, s]
                        b_s = bt[:, s]
                        u = uvpool.tile([128, B], F16, tag="u")
                        # u = cab*a + cb  (ACT scale/bias)
                        nc.scalar.activation(u[:], a_s, ident,
                                             bias=cb_32[:, t:t + 1],
                                             scale=cab_32[:, t:t + 1])
                        # w = u*b  (DVE tensor_tensor, 2x fp16)
                        nc.vector.tensor_tensor(u[:], u[:], b_s, op=mult)
                        if s % 2 == 0:
                            # R: v on ACT, final add on DVE (tt, 2x fp16)
                            v = uvpool.tile([128, B], F16, tag="v")
                            nc.scalar.activation(v[:], a_s, ident,
                                                 bias=c0_32[:, t:t + 1],
                                                 scale=ca_32[:, t:t + 1])
                            nc.vector.tensor_tensor(ot[:, s], u[:], v[:],
                                                    op=add)
                        else:
                            # Q: fused (a*ca + c0) + w on DVE
                            nc.vector.affine_then_add(
                                ot[:, s], a_s, u[:],
                                scale=ca_32[:, t:t + 1],
                                bias=c0_32[:, t:t + 1])
                    nc.sync.dma_start(out_r[ci], ot[:])

    nc.compile()
    return nc


_PROGRAM = None


def _get_program():
    global _PROGRAM
    if _PROGRAM is None:
        _PROGRAM = build_program()
    return _PROGRAM


def _coeff_tensors(weights):
    """softmax(weights) @ OP_COEFFS -> [128, 4, NT] f32 device layout."""
    w = weights.astype(np.float32)
    e = np.exp(w - w.max(axis=1, keepdims=True))
    sm = e / e.sum(axis=1, keepdims=True)
    coef = sm @ _OP_COEFFS                      # [OUT_DIM, 4]
    # (p, m, t) = coef[t*128 + p, m]
    c = coef.reshape(NT, 128, 4).transpose(1, 2, 0)
    return np.ascontiguousarray(c, dtype=np.float32)


def _wrap_idx(idx):
    """[OUT_DIM] int -> SWDGE-wrapped int16 [128, OUT_DIM//16].

    Per NJC-chunk c, columns [c*NJC//16:(c+1)*NJC//16] hold that chunk's
    indices with index i at (partition i%16, column i//16), replicated
    across the 8 groups of 16 partitions (one per Q7 core).
    """
    i16 = idx.astype(np.int16).reshape(NCH, NJC // 16, 16)
    w = i16.transpose(2, 0, 1).reshape(16, NCH * (NJC // 16))
    return np.ascontiguousarray(np.tile(w, (8, 1)))


def prepare_in_maps(x, weights, idx_a, idx_b):
    x = np.asarray(x, dtype=np.float32)
    weights = np.asarray(weights, dtype=np.float32)
    cf32 = _coeff_tensors(weights)
    ia = _wrap_idx(np.asarray(idx_a))
    ib = _wrap_idx(np.asarray(idx_b))

    in_maps = []
    for c in range(NCORES):
        xt = np.ascontiguousarray(x[c * B:(c + 1) * B].T).astype(np.float16)
        in_maps.append({"xt": xt, "cf32": cf32, "idxa": ia, "idxb": ib})
    return in_maps


def assemble_output(results):
    out = np.empty((BATCH, OUT_DIM), dtype=np.float32)
    for c in range(NCORES):
        # scratch [NCH, 128, SL, B] -> [B, NCH*SL*128] with j = ci*NJC+s*128+p
        scr = results[c]["out"]
        out[c * B:(c + 1) * B] = (
            scr.transpose(3, 0, 2, 1).reshape(B, OUT_DIM).astype(np.float32))
    return out


def kernel(x, weights, idx_a, idx_b):
    nc = _get_program()
    in_maps = prepare_in_maps(x, weights, idx_a, idx_b)
    res = run_bass_kernel_spmd(nc, in_maps, list(range(NCORES)))
    return assemble_output(res.results)
